# revision 1
# baseline (speedup 1.0000x reference)
"""Trainium2 Bass kernel for an 8-branch MLP block with layernorm + gelu + skip.

Reference computation (per branch n of 8, batch B=16384, vocab D=256, ffn E=1024):
    h   = gelu_exact(x[:, n, :] @ U1[n] + b1[n])          # (B, E)
    y   = h @ U2[n] + b2[n]                               # (B, D)
    z   = layernorm(y) * ln_w + ln_b
    out[:, n, :] = x[:, n, :] + gelu_exact(z)
Output reshaped to (B, 1, 8*D).

Sharding: expert-parallel — branch n on NeuronCore n (8 cores). Each core's
matmul path runs in float32r (full PE speed at free-dim >= 256, ~1e-4 rel
err); layernorm statistics, skip-add and output are fp32.
"""

import numpy as np

BATCH, BRANCH, VOCAB, FFN = 16384, 8, 256, 1024
LN_EPS = 1e-5
BLK = 512  # batch rows per pipeline block
NBLK = BATCH // BLK
NBC = BLK // 128  # 128-row chunks per block
NKC = VOCAB // 128  # contraction chunks for GEMM1
NEC = FFN // 128  # e-chunks (GEMM1 output tiles / GEMM2 contraction)

_CACHE = {}
LAST_EXEC_NS = None


def _build(general_ln: bool, reps: int = 1):
    import concourse.bacc as bacc
    import concourse.tile as tile
    import concourse.mybir as mybir

    f32 = mybir.dt.float32
    f32r = mybir.dt.float32r
    Act = mybir.ActivationFunctionType

    nc = bacc.Bacc(None, target_bir_lowering=False)

    xt = nc.dram_tensor("xt", [VOCAB, BATCH], f32r, kind="ExternalInput")
    xb = nc.dram_tensor("xb", [BATCH, VOCAB], f32, kind="ExternalInput")
    u1 = nc.dram_tensor("u1", [VOCAB, FFN], f32r, kind="ExternalInput")
    u2 = nc.dram_tensor("u2", [FFN, VOCAB], f32r, kind="ExternalInput")
    b1r = nc.dram_tensor("b1r", [128, NEC], f32, kind="ExternalInput")
    b2bc = nc.dram_tensor("b2bc", [128, NBC, VOCAB], f32, kind="ExternalInput")
    if general_ln:
        lnwbc = nc.dram_tensor("lnwbc", [128, NBC, VOCAB], f32, kind="ExternalInput")
        lnbbc = nc.dram_tensor("lnbbc", [128, NBC, VOCAB], f32, kind="ExternalInput")
    out = nc.dram_tensor("out", [BATCH, VOCAB], f32, kind="ExternalOutput")

    with tile.TileContext(nc) as tc:
        with (
            tc.tile_pool(name="singles", bufs=1) as singles,
            tc.tile_pool(name="xtp", bufs=4) as xtp,
            tc.tile_pool(name="xbp", bufs=3) as xbp,
            tc.tile_pool(name="hp", bufs=2) as hp,
            tc.tile_pool(name="yp", bufs=3) as yp,
            tc.tile_pool(name="sp", bufs=8) as sp,
            tc.tile_pool(name="op", bufs=3) as op,
            tc.tile_pool(name="phq", bufs=4, space="PSUM") as phq,
            tc.tile_pool(name="pyq", bufs=2, space="PSUM") as pyq,
        ):
            # --- resident weights / constants. Ordered so the first GEMM1
            # matmul (needs u1[kc=0] + xt0[kc=0]) can start as early as
            # possible ---
            u1_t = singles.tile([128, NKC, FFN], f32r)
            xt0_t = xtp.tile([128, NKC, BLK], f32r, tag="xt")
            u1_view = u1.rearrange("(c p) e -> p c e", p=128)
            xt0_view = xt[:, 0:BLK].rearrange("(c p) m -> p c m", p=128)
            nc.sync.dma_start(u1_t[:, 0, 0:128], u1_view[:, 0, 0:128])
            nc.sync.dma_start(xt0_t[:, 0, :], xt0_view[:, 0, :])
            nc.sync.dma_start(u1_t[:, 0, 128:FFN], u1_view[:, 0, 128:FFN])
            nc.sync.dma_start(xt0_t[:, 1, :], xt0_view[:, 1, :])
            nc.sync.dma_start(u1_t[:, 1, :], u1_view[:, 1, :])
            u2_t = singles.tile([128, NEC, VOCAB], f32r)
            u2_view = u2.rearrange("(c p) d -> p c d", p=128)
            nc.sync.dma_start(u2_t[:, 0:2, :], u2_view[:, 0:2, :])
            b1_t = singles.tile([128, NEC], f32)
            nc.sync.dma_start(b1_t[:], b1r[:])
            nc.sync.dma_start(u2_t[:, 2:NEC, :], u2_view[:, 2:NEC, :])

            def prefetch_xt(it):
                i = it % NBLK
                t = xtp.tile([128, NKC, BLK], f32r, tag="xt")
                nc.sync.dma_start(
                    t[:],
                    xt[:, i * BLK : (i + 1) * BLK].rearrange(
                        "(c p) m -> p c m", p=128
                    ),
                )
                return t

            xt_queue = [xt0_t]
            for j in range(1, min(3, NBLK * reps)):
                xt_queue.append(prefetch_xt(j))

            b2_t = singles.tile([128, NBC, VOCAB], f32)
            if general_ln:
                lnw_t = singles.tile([128, NBC, VOCAB], f32)
                nc.sync.dma_start(lnw_t[:], lnwbc[:])
                lnb_t = singles.tile([128, NBC, VOCAB], f32)
                nc.sync.dma_start(lnb_t[:], lnbbc[:])
            magic_t = singles.tile([128, NBC], mybir.dt.int32)
            nc.vector.memset(magic_t[:], 0x5F3759DF)
            # dummy activation: pull the Gelu LUT load into the startup DMA
            # window instead of stalling the first real gelu1
            warm_t = singles.tile([128, 1], f32)
            nc.vector.memset(warm_t[:], 0.0)
            nc.scalar.activation(warm_t[:], warm_t[:], Act.Gelu)
            # dummy matmuls: spend the HAM clock-gate warmup (~3.4us of PE
            # busy before 2.4GHz) inside the startup DMA window on zeroed data
            warm_w = singles.tile([128, 128], f32r)
            nc.vector.memset(warm_w[:].bitcast(f32), 0.0)
            warm_r = singles.tile([128, VOCAB], f32r)
            nc.vector.memset(warm_r[:].bitcast(f32), 0.0)
            warm_ps = phq.tile([128, BLK], f32, tag="ph")
            for _ in range(16):
                nc.tensor.matmul(
                    warm_ps[:, 0:VOCAB], warm_w[:], warm_r[:], start=True, stop=True
                )

            # GEMM2 runs one full block behind GEMM1 (software pipeline): its
            # h-tiles were produced a block earlier, so no matmul ever waits
            # on an activation. Groups are bc-outer: within a PSUM bank only
            # one accumulation group is open at a time (start=True clears
            # has_written for the WHOLE bank).

            def epilogue(bs, py, xb_t, fused=False, bc0=0, nbc=NBC, py_bc0=None,
                         pool_skip=False):
                # +b2, layernorm stats, gelu, skip add. Emitted one block late
                # so the ACT stream orders gelu1(i+1) before gelu2(i) and the
                # DVE chain never blocks the next block's activations.
                bsl = slice(bc0, bc0 + nbc)
                if py_bc0 is None:
                    py_bc0 = bc0
                yb = yp.tile([128, nbc, VOCAB], f32, tag="yb")
                nc.vector.tensor_add(
                    yb[:], py[:, py_bc0 : py_bc0 + nbc, :], b2_t[:, bsl, :]
                )
                mvs = sp.tile([128, nbc, 2], f32, tag="mvs")
                stats = sp.tile([128, nbc, 6], f32, tag="stats")
                for bc in range(nbc):
                    nc.vector.bn_stats(stats[:, bc, :], yb[:, bc, :])
                for bc in range(nbc):
                    nc.vector.bn_aggr(mvs[:, bc, :], stats[:, bc, :])
                # rstd = rsqrt(var + eps) via bit-trick + 2 Newton steps (DVE;
                # ACT Sqrt would thrash the activation table against Gelu).
                # 2 steps -> ~5e-6 rel err, far below the f32r matmul noise.
                ve = sp.tile([128, nbc], f32, tag="ve")
                nc.vector.tensor_scalar(
                    out=ve[:], in0=mvs[:, :, 1], scalar1=LN_EPS, scalar2=None,
                    op0=mybir.AluOpType.add,
                )
                yi = sp.tile([128, nbc], mybir.dt.int32, tag="yi")
                nc.vector.tensor_scalar(
                    out=yi[:], in0=ve[:].bitcast(mybir.dt.int32), scalar1=1,
                    scalar2=None, op0=mybir.AluOpType.arith_shift_right,
                )
                rstd = sp.tile([128, nbc], f32, tag="rstd")
                nc.vector.tensor_sub(
                    rstd[:].bitcast(mybir.dt.int32), magic_t[:, 0:nbc], yi[:]
                )
                nt1 = sp.tile([128, nbc], f32, tag="nt1")
                nt2 = sp.tile([128, nbc], f32, tag="nt2")
                for _ in range(2):
                    nc.vector.tensor_mul(nt1[:], rstd[:], rstd[:])
                    nc.vector.tensor_mul(nt2[:], nt1[:], ve[:])
                    nc.vector.tensor_scalar(
                        out=nt2[:], in0=nt2[:], scalar1=-0.5, scalar2=1.5,
                        op0=mybir.AluOpType.mult, op1=mybir.AluOpType.add,
                    )
                    nc.vector.tensor_mul(rstd[:], nt2[:], rstd[:])
                out_view = out[bs : bs + BLK, :].rearrange(
                    "(c p) d -> p c d", p=128
                )[:, bsl, :]
                if fused and not general_ln:
                    # tail blocks: fuse scale/bias into per-bc ACT gelu and
                    # pipeline per-bc skip-add + store to shorten the serial
                    # drain chain (no later gelu1 competes for ACT here)
                    nmr = sp.tile([128, nbc], f32, tag="nmr")
                    nc.vector.tensor_mul(nmr[:], mvs[:, :, 0], rstd[:])
                    nc.vector.tensor_scalar(
                        out=nmr[:], in0=nmr[:], scalar1=-1.0, scalar2=None,
                        op0=mybir.AluOpType.mult,
                    )
                    g_t = op.tile([128, nbc, VOCAB], f32, tag="g")
                    o_t = op.tile([128, nbc, VOCAB], f32, tag="o")
                    adder = nc.gpsimd if pool_skip else nc.vector
                    for bc in range(nbc):
                        nc.scalar.activation(
                            g_t[:, bc, :], yb[:, bc, :], Act.Gelu,
                            bias=nmr[:, bc : bc + 1], scale=rstd[:, bc : bc + 1],
                        )
                        adder.tensor_add(
                            o_t[:, bc, :], g_t[:, bc, :], xb_t[:, bc0 + bc, :]
                        )
                        nc.sync.dma_start(out_view[:, bc, :], o_t[:, bc, :])
                    return
                # z = (y - mu) * rstd on DVE (per-partition scalars), then one
                # batched Gelu on ACT — keeps ACT well under the PE's budget
                z_t = op.tile([128, nbc, VOCAB], f32, tag="z")
                for bc in range(nbc):
                    nc.vector.tensor_scalar(
                        out=z_t[:, bc, :], in0=yb[:, bc, :],
                        scalar1=mvs[:, bc, 0:1], scalar2=rstd[:, bc : bc + 1],
                        op0=mybir.AluOpType.subtract, op1=mybir.AluOpType.mult,
                    )
                if general_ln:
                    nc.vector.tensor_mul(z_t[:], z_t[:], lnw_t[:, bsl, :])
                    nc.vector.tensor_add(z_t[:], z_t[:], lnb_t[:, bsl, :])
                g_t = op.tile([128, nbc, VOCAB], f32, tag="g")
                nc.scalar.activation(g_t[:], z_t[:], Act.Gelu)
                o_t = op.tile([128, nbc, VOCAB], f32, tag="o")
                adder = nc.gpsimd if pool_skip else nc.vector
                adder.tensor_add(o_t[:], g_t[:], xb_t[:, bsl, :])
                nc.sync.dma_start(out_view[:], o_t[:])

            def gemm2_mms(h_prev, py):
                # flat list of GEMM2 matmuls for one block, bc-outer
                mms = []
                for bc in range(NBC):
                    for ec in range(NEC):
                        mms.append(
                            lambda bc=bc, ec=ec: nc.tensor.matmul(
                                py[:, bc, :],
                                h_prev[:, ec, bc * 128 : (bc + 1) * 128],
                                u2_t[:, ec, :],
                                start=(ec == 0),
                                stop=(ec == NEC - 1),
                            )
                        )
                return mms

            g2_prev = None  # (bs, h_t, xb_t) of block i-1, G2 still to emit
            pending_ep = None  # (bs, py, xb_t) of block i-2, epilogue to emit

            for it in range(NBLK * reps):
                i = it % NBLK
                bs = i * BLK
                # activations for this block, feature-major (contraction on
                # partitions). xt is prefetched two blocks ahead (FIFO).
                xt_t = xt_queue.pop(0)
                if it + 3 <= NBLK * reps - 1:
                    xt_queue.append(prefetch_xt(it + 3))

                if it == 1:
                    # b2 constants are first needed by ep(0) during block 2 —
                    # emitted here so early xt prefetches win the DMA queue
                    nc.sync.dma_start(b2_t[:], b2bc[:])
                h_t = hp.tile([128, NEC, BLK], f32r)
                if g2_prev is not None:
                    bs_p, h_prev, xb_prev = g2_prev
                    py = pyq.tile([128, NBC, VOCAB], f32, tag="py")
                    g2 = gemm2_mms(h_prev, py)
                else:
                    py = g2 = None

                # On the final block, run all of G2(i-1) first: py(i-1)
                # completes ~3.4us earlier so its epilogue's DVE chain
                # overlaps the remaining PE work instead of draining serially.
                last = it == NBLK * reps - 1
                if last and g2 is not None:
                    for mm in g2:
                        mm()
                for ec in range(NEC):
                    ph = phq.tile([128, BLK], f32)
                    for kc in range(NKC):
                        nc.tensor.matmul(
                            ph[:],
                            u1_t[:, kc, ec * 128 : (ec + 1) * 128],
                            xt_t[:, kc, :],
                            start=(kc == 0),
                            stop=(kc == NKC - 1),
                        )
                    nc.scalar.activation(
                        h_t[:, ec, :], ph[:], Act.Gelu, bias=b1_t[:, ec : ec + 1]
                    )
                    if g2 is not None and not last:
                        for mm in g2[ec * NBC : (ec + 1) * NBC]:
                            mm()

                # batch-major rows for the skip connection (needed by this
                # block's epilogue — emitted after the matmuls so the DMA
                # queue prioritizes xt prefetch)
                xb_t = xbp.tile([128, NBC, VOCAB], f32)
                nc.sync.dma_start(
                    xb_t[:], xb[bs : bs + BLK, :].rearrange("(c p) d -> p c d", p=128)
                )

                if pending_ep is not None:
                    epilogue(*pending_ep, pool_skip=last)
                    pending_ep = None
                if g2 is not None:
                    pending_ep = (bs_p, py, xb_prev)
                g2_prev = (bs, h_t, xb_t)

            # flush: GEMM2 of the last block, split across two separate PSUM
            # tiles so the first half's epilogue (tile-granular dependency)
            # overlaps the second half's matmuls, shortening the serial drain.
            bs_p, h_prev, xb_prev = g2_prev
            py_a = pyq.tile([128, 2, VOCAB], f32, tag="py")
            py_b = pyq.tile([128, 2, VOCAB], f32, tag="py")
            halves = []
            for half, py_h in ((0, py_a), (1, py_b)):
                for bc in range(2):
                    for ec in range(NEC):
                        halves.append(
                            lambda half=half, bc=bc, ec=ec, py_h=py_h: nc.tensor.matmul(
                                py_h[:, bc, :],
                                h_prev[:, ec, (half * 2 + bc) * 128 : (half * 2 + bc + 1) * 128],
                                u2_t[:, ec, :],
                                start=(ec == 0),
                                stop=(ec == NEC - 1),
                            )
                        )
            for mm in halves[: 2 * NEC]:
                mm()
            if pending_ep is not None:
                epilogue(*pending_ep, fused=True, pool_skip=True)
            for mm in halves[2 * NEC :]:
                mm()
            epilogue(bs_p, py_a, xb_prev, fused=True, bc0=0, nbc=2, py_bc0=0, pool_skip=True)
            epilogue(bs_p, py_b, xb_prev, fused=True, bc0=2, nbc=2, py_bc0=0, pool_skip=True)

    nc.compile()
    return nc


def _get_nc(general_ln: bool, reps: int = 1):
    key = ("nc", general_ln, reps)
    if key not in _CACHE:
        _CACHE[key] = _build(general_ln, reps)
    return _CACHE[key]


def kernel(x, U1, b1, U2, b2, ln_w, ln_b):
    global LAST_EXEC_NS
    from concourse.bass_utils import run_bass_kernel_spmd

    x = np.asarray(x, dtype=np.float32)
    U1 = np.asarray(U1, dtype=np.float32)
    b1 = np.asarray(b1, dtype=np.float32)
    U2 = np.asarray(U2, dtype=np.float32)
    b2 = np.asarray(b2, dtype=np.float32)
    ln_w = np.asarray(ln_w, dtype=np.float32)
    ln_b = np.asarray(ln_b, dtype=np.float32)

    general_ln = not (
        np.all(ln_w == np.float32(1.0)) and np.all(ln_b == np.float32(0.0))
    )
    nc = _get_nc(general_ln)

    in_maps = []
    for n in range(BRANCH):
        xb_n = np.ascontiguousarray(x[:, n, :])
        m = {
            "xt": np.ascontiguousarray(xb_n.T),
            "xb": xb_n,
            "u1": np.ascontiguousarray(U1[n]),
            "u2": np.ascontiguousarray(U2[n]),
            "b1r": np.ascontiguousarray(b1[n].reshape(NEC, 128).T),
            "b2bc": np.broadcast_to(
                b2[n], (128, NBC, VOCAB)
            ).copy(),
        }
        if general_ln:
            m["lnwbc"] = np.broadcast_to(ln_w, (128, NBC, VOCAB)).copy()
            m["lnbbc"] = np.broadcast_to(ln_b, (128, NBC, VOCAB)).copy()
        in_maps.append(m)

    res = run_bass_kernel_spmd(nc, in_maps, core_ids=list(range(BRANCH)))
    LAST_EXEC_NS = res.exec_time_ns

    outp = np.empty((BATCH, BRANCH, VOCAB), dtype=np.float32)
    for n in range(BRANCH):
        outp[:, n, :] = res.results[n]["out"]
    return outp.reshape(BATCH, 1, BRANCH * VOCAB)

